# revision 3
# baseline (speedup 1.0000x reference)
"""DigitCaps dynamic-routing kernel for 8x Trainium2 NeuronCores.

Full inputs -> batch-sharded across 8 cores (16 samples/core), W replicated.

Per-core layout:
  u_hat[(r_l,b)=128 partitions, g=256, o=16, c=10]  (bf16 in SBUF)
    where route r = g*8 + r_l   (8 routes per matmul group)
  b_ij / c_ij: (128p, 256g, 10c)   (partition carries (r_l, b))

u_hat build: per group g, one PE matmul:
  lhsT = Xblk[g] (K=64=(r_l,i), M=128=(r_l,b))  block-diagonal x (host-built)
  rhs  = Wt[g]   (K=64=(r_l,i), N=160=(o,c))
  out  = psum (128=(r_l,b), 160=(o,c)) -> drain to SBUF as bf16
  PSUM drains alternate ACT/DVE; iteration-1's s-sum matmuls (c==0.1
  exactly -> s1 = 0.1*sum_r u_hat) are folded into phase 1, accumulating
  into a dedicated PSUM bank as groups drain.

s_j = sum_r c*u_hat: PE matmuls with lhsT = tile(eye(16),(8,8)):
  sums over partitions (r_l) while replicating the (16b, 160) result to all
  128 partitions; 3 groups per matmul accumulated into one PSUM tile.

Iterations 2..3: per 32-group chunk the elementwise work is split between
DVE (28 groups) and GpSimd (4 groups) for the big multiplies/tree, with
GpSimd also taking the b_ij update and c normalization multiply; ACT does
exp. b_ij/c_exp stored bf16 (errors wash out over 2048 routes).
"""

import sys

for p in ("/opt/trn_rl_repo",):
    if p not in sys.path:
        sys.path.insert(0, p)

import numpy as np
import ml_dtypes

import concourse.bass as bass
import concourse.bacc as bacc
import concourse.mybir as mybir
import concourse.tile as tile
from concourse.bass_utils import run_bass_kernel_spmd

# Problem constants (hardcoded per contract)
B_FULL = 128
N_CORES = 8
B = B_FULL // N_CORES  # 16 samples per core
R = 2048
C = 10
O = 16
I = 8
ITERS = 3

RG = 8               # routes per matmul group
G = R // RG          # 256 groups
K = RG * I           # 64 contraction rows per group
CO = C * O           # 160
CH = 32              # groups per routing chunk
NCH = G // CH        # 16 chunks
GP = 4               # groups per chunk handled by GpSimd (rest on DVE)
GD = CH - GP         # DVE groups per chunk
CPAD = 12            # padded capsule dim for 4B alignment of bf16 rows

F32 = mybir.dt.float32
BF16 = mybir.dt.bfloat16

_COMPILED = None  # cache (nc, names) across calls


def _host_prep(x, W):
    """Build per-core DMA-ready arrays. x: (128,2048,8) W: (2048,10,16,8)."""
    x = np.ascontiguousarray(x, dtype=np.float32)
    W = np.ascontiguousarray(W, dtype=np.float32)

    # Wt[g, r_l*8+i, o*10+c] = W[g*8+r_l, c, o, i]
    Wt = W.transpose(0, 3, 2, 1).reshape(G, RG, I, O, C).reshape(G, K, CO)
    # interleave for the build layout: per 8-group sub-chunk, group
    # g0+two*4+gp -> partitions two*64..+63, free slot gp; then pair
    # sub-chunks into 16-group DMA chunks
    Wt8 = (Wt.reshape(G // 8, 2, 4, K, CO).transpose(0, 1, 3, 2, 4)
           .reshape(G // 8, 128, 4, CO))
    Wt16 = (Wt8.reshape(G // 16, 2, 128, 4, CO).transpose(0, 2, 1, 3, 4)
            .reshape(G // 16, 128, 8, CO))

    # Bmask[(r_l,b), (r_l',b')] = 1 if b==b'  -> psum = sum over r_l,
    # replicated across all output partitions
    bmask = np.tile(np.eye(B, dtype=np.float32), (RG, RG))  # (128, 128)
    bmask_bf = bmask.astype(np.float32)

    shards = []
    for ci in range(N_CORES):
        xs = x[ci * B : (ci + 1) * B]  # (16, 2048, 8)
        # xt[g, r_l, i, b] = xs[b, g*8+r_l, i]
        xt = xs.transpose(1, 2, 0).reshape(G, RG, I, B)
        # Block-diagonal lhsT: Xblk[g, r_l*8+i, r_l*16+b] = xt[g, r_l, i, b]
        xblk = np.zeros((G, RG, I, RG, B), dtype=np.float32)
        idx = np.arange(RG)
        xblk[:, idx, :, idx, :] = xt.transpose(1, 0, 2, 3)
        xblk = xblk.reshape(G, K, RG * B)
        xb8 = (xblk.reshape(G // 8, 2, 4, K, RG * B).transpose(0, 1, 3, 2, 4)
               .reshape(G // 8, 128, 4, RG * B))
        xb16 = (xb8.reshape(G // 16, 2, 128, 4, RG * B)
                .transpose(0, 2, 1, 3, 4).reshape(G // 16, 128, 8, RG * B))
        xwt = np.concatenate([xb16, Wt16], axis=3)  # (16, 128, 8, 288)
        shards.append(np.ascontiguousarray(xwt).astype(ml_dtypes.bfloat16))
    return shards, bmask_bf


def _build_kernel():
    nc = bacc.Bacc("TRN2", target_bir_lowering=False, debug=False,
                   num_devices=N_CORES)

    xwt_d = nc.dram_tensor("xwt", [G // 16, 128, 8, 128 + CO], BF16,
                           kind="ExternalInput")
    bmask_d = nc.dram_tensor("bmask", [128, 128], F32, kind="ExternalInput")
    vout_d = nc.dram_tensor("vout", [B, O, C], F32, kind="ExternalOutput")

    with tile.TileContext(nc) as tc:
        with (
            tc.tile_pool(name="persist", bufs=1) as persist,
            tc.tile_pool(name="xw", bufs=3) as xw,
            tc.tile_pool(name="work", bufs=6) as work,
            tc.tile_pool(name="psum", bufs=6, space="PSUM") as psum,
            tc.tile_pool(name="spsum", bufs=2, space="PSUM") as spsum,
        ):
            uhat = persist.tile([128, G, O, C], BF16)      # 80 KiB/part
            bij = persist.tile([128, G, C], BF16)          # 5 KiB
            cexp = persist.tile([128, G, C], BF16)         # 5 KiB
            cbf = persist.tile([128, G, CPAD], BF16)       # 6 KiB
            bmask = persist.tile([128, 128], F32)
            bmask_b = persist.tile([128, 128], BF16)
            v_bf = persist.tile([128, O, C], BF16)
            s_sb = persist.tile([128, O, C], F32)
            sq = persist.tile([128, C], F32)
            sq2 = persist.tile([128, C], F32)
            zsum = persist.tile([128, G], F32)
            zrec = persist.tile([128, G], BF16)
            eps_t = persist.tile([128, 1], F32)
            nc.gpsimd.memset(eps_t[:], 1e-8)

            nc.sync.dma_start(bmask[:], bmask_d[:])
            nc.vector.tensor_copy(bmask_b[:], bmask[:])

            # ---------- Phase 1: u_hat build + iter-1 s accumulation ----
            # Group pairs (g0+j, g0+4+j) run concurrently in PE row-groups
            # 0-63 / 64-127 (K=64 each); their outputs go to different PSUM
            # banks so the row-tiles don't serialize on the bank tracker.
            # One 576KiB DMA per 16-group chunk; within: 2 sub-chunks of 8
            # groups. Group g0+two*4+gp -> partitions two*64..+63, free slot
            # (sub*4+gp); xb = slot[..., 0:128], wt = slot[..., 128:288].
            # As each 16-group chunk is drained, its contribution to
            # s1 = 0.1*sum_r u_hat accumulates into the sp PSUM tile (the
            # iteration-1 c_ij are exactly 0.1).
            sp = spsum.tile([128, 3, O, C], F32, tag="sp")
            for dc in range(G // 16):
                xwt_t = xw.tile([128, 8, 128 + CO], BF16, tag="xwt")
                eng = nc.sync if dc % 2 == 0 else nc.scalar
                eng.dma_start(xwt_t[:], xwt_d[dc])
                for sub in range(2):
                    g0 = dc * 16 + sub * 8
                    pst = [psum.tile([128, 2, O, C], F32, tag="ps",
                                     name=f"ps{g0}_{t}") for t in range(4)]
                    for j in range(4):
                        for two in range(2):
                            g = g0 + two * 4 + j
                            lo = (g - g0) % 8
                            sl = xwt_t[two * 64 : two * 64 + 64, sub * 4 + j]
                            nc.tensor.matmul(
                                pst[lo // 2][:, lo & 1],
                                lhsT=sl[:, 0:128],
                                rhs=sl[:, 128 : 128 + CO],
                                start=True, stop=True)
                    for t in range(4):
                        if t % 2 == 0:
                            nc.scalar.copy(
                                uhat[:, g0 + 2 * t : g0 + 2 * t + 2], pst[t][:])
                        else:
                            nc.vector.tensor_copy(
                                uhat[:, g0 + 2 * t : g0 + 2 * t + 2], pst[t][:])
                # iter-1 s-sum over this chunk's 16 groups: 5 triples + 1
                gc = dc * 16
                for m in range(5):
                    nc.tensor.matmul(
                        sp[:], lhsT=bmask_b[:],
                        rhs=uhat[:, gc + 3 * m : gc + 3 * m + 3].rearrange(
                            "p g o c -> p (g o c)"),
                        start=(dc == 0 and m == 0), stop=False)
                nc.tensor.matmul(
                    sp[:, 0], lhsT=bmask_b[:],
                    rhs=uhat[:, gc + 15].rearrange("p o c -> p (o c)"),
                    start=False, stop=(dc == G // 16 - 1))

            # ---------- helpers ----------
            def s_combine(sp, scale):
                # s_sb = (sp0 + sp1 + sp2) * scale; only one PSUM read per op
                nc.scalar.copy(s_sb[:], sp[:, 0])
                nc.vector.tensor_add(s_sb[:], s_sb[:], sp[:, 1])
                nc.vector.tensor_add(s_sb[:], s_sb[:], sp[:, 2])
                if scale != 1.0:
                    nc.scalar.mul(s_sb[:], s_sb[:], scale)

            # ---------- iteration 1 (s already accumulated) ----------
            s_combine(sp, 0.1)
            _squash(nc, work, s_sb, sq, sq2, v_bf, eps_t)

            # ---------- iterations 2..3 ----------
            # Per-chunk pipeline split across engines: DVE handles 28 of 32
            # groups of the big multiplies + o-tree, GpSimd the other 4 plus
            # the b_ij update and c normalization; ACT does exp. The product
            # for chunk ch+1 is issued before chunk ch's softmax chain so
            # DVE/GpSimd never idle on the cross-engine handoffs.
            for it in range(1, ITERS):
                sp = spsum.tile([128, 3, O, C], F32, tag="sp",
                                name=f"sp_{it}")

                def agree_mult(ch):
                    g0 = ch * CH
                    pa = work.tile([128, CH, O, C], BF16, tag="prod",
                                   name=f"pa{it}_{ch}")
                    nc.gpsimd.tensor_mul(
                        pa[:, GD:CH], uhat[:, g0 + GD : g0 + CH],
                        v_bf[:].unsqueeze(1).broadcast_to((128, GP, O, C)))
                    nc.vector.tensor_mul(
                        pa[:, 0:GD], uhat[:, g0 : g0 + GD],
                        v_bf[:].unsqueeze(1).broadcast_to((128, GD, O, C)))
                    # o-reduction tree, each engine on its own slice
                    for (lo, hi) in ((8, 16), (4, 8), (2, 4)):
                        nc.gpsimd.tensor_add(
                            pa[:, GD:CH, 0:lo], pa[:, GD:CH, 0:lo],
                            pa[:, GD:CH, lo:hi])
                        nc.vector.tensor_add(
                            pa[:, 0:GD, 0:lo], pa[:, 0:GD, 0:lo],
                            pa[:, 0:GD, lo:hi])
                    return pa

                pa = agree_mult(0)
                for ch in range(NCH):
                    g0 = ch * CH
                    sl = slice(g0, g0 + CH)
                    # b_ij update (GpSimd, full chunk)
                    if it == 1:
                        nc.gpsimd.tensor_add(bij[:, sl],
                                             pa[:, :, 0], pa[:, :, 1])
                    else:
                        nc.gpsimd.tensor_add(pa[:, :, 0], pa[:, :, 0],
                                             pa[:, :, 1])
                        nc.gpsimd.tensor_add(bij[:, sl],
                                             bij[:, sl], pa[:, :, 0])
                    # chunk-local softmax over c (exp on ACT)
                    nc.scalar.activation(cexp[:, sl], bij[:, sl],
                                         mybir.ActivationFunctionType.Exp)
                    pa_next = agree_mult(ch + 1) if ch + 1 < NCH else None
                    nc.vector.reduce_sum(zsum[:, sl], cexp[:, sl],
                                         axis=mybir.AxisListType.X)
                    with nc.allow_low_precision(
                            reason="per-route softmax scale; bf16 error is "
                            "a common factor per route and washes out over "
                            "the 2048-route sum"):
                        nc.vector.reciprocal(zrec[:, sl], zsum[:, sl])
                    nc.gpsimd.tensor_mul(
                        cbf[:, sl, 0:C], cexp[:, sl],
                        zrec[:, sl].unsqueeze(2).broadcast_to((128, CH, C)))
                    # s products + PE group-triple sums
                    prods = work.tile([128, CH, O, C], BF16, tag="prod",
                                      name=f"psx{it}_{ch}")
                    nc.gpsimd.tensor_mul(
                        prods[:, GD:CH], uhat[:, g0 + GD : g0 + CH],
                        cbf[:, g0 + GD : g0 + CH, 0:C].unsqueeze(2)
                        .broadcast_to((128, GP, O, C)))
                    nc.vector.tensor_mul(
                        prods[:, 0:GD], uhat[:, g0 : g0 + GD],
                        cbf[:, g0 : g0 + GD, 0:C].unsqueeze(2)
                        .broadcast_to((128, GD, O, C)))
                    # 10 triples + one pair per 32-group chunk
                    for j in range(10):
                        nc.tensor.matmul(
                            sp[:], lhsT=bmask_b[:],
                            rhs=prods[:, 3 * j : 3 * j + 3].rearrange(
                                "p g o c -> p (g o c)"),
                            start=(ch == 0 and j == 0), stop=False)
                    nc.tensor.matmul(
                        sp[:, 0:2], lhsT=bmask_b[:],
                        rhs=prods[:, 30:32].rearrange("p g o c -> p (g o c)"),
                        start=False, stop=(ch == NCH - 1))
                    pa = pa_next
                s_combine(sp, 1.0)
                _squash(nc, work, s_sb, sq, sq2, v_bf, eps_t)

            # ---------- output ----------
            vfin = work.tile([128, O, C], F32, tag="vfin")
            nc.vector.tensor_mul(
                vfin[:], s_sb[:],
                sq[:].unsqueeze(1).broadcast_to((128, O, C)))
            nc.sync.dma_start(vout_d[:], vfin[0:B])

    nc.compile()
    return nc


def _squash(nc, work, s_sb, sq, sq2, v16, eps_t):
    """v = s * (|s|^2/(1+|s|^2)) / sqrt(|s|^2 + 1e-8), per (b, c).

    Runs on B=16 partitions; leaves the scale factor in `sq`;
    v16 = s * scale (bf16). s_sb layout (B, O, C).
    """
    P = s_sb.shape[0]
    ssq = work.tile([P, O, C], F32, tag="ssq")
    nc.vector.tensor_mul(ssq[:], s_sb[:], s_sb[:])
    nc.vector.reduce_sum(sq[:], ssq[:].rearrange("p o c -> p c o"),
                         axis=mybir.AxisListType.X)
    # sq2 = (1+n)*sqrt(n+1e-8);  sq = n / sq2
    nc.scalar.activation(sq2[:], sq[:], mybir.ActivationFunctionType.Sqrt,
                         bias=eps_t[0:P])
    nc.vector.scalar_tensor_tensor(
        sq2[:], sq[:], 1.0, sq2[:],
        op0=mybir.AluOpType.add, op1=mybir.AluOpType.mult)
    nc.vector.reciprocal(sq2[:], sq2[:])
    nc.vector.tensor_mul(sq[:], sq[:], sq2[:])
    nc.vector.tensor_mul(
        v16[:], s_sb[:], sq[:].unsqueeze(1).broadcast_to((P, O, C)))


def kernel(x, W):
    global _COMPILED
    xshards, bmask = _host_prep(x, W)
    if _COMPILED is None:
        _COMPILED = _build_kernel()
    nc = _COMPILED
    in_maps = [
        {"xwt": xs, "bmask": bmask} for xs in xshards
    ]
    res = run_bass_kernel_spmd(nc, in_maps, list(range(N_CORES)))
    outs = []
    for ci in range(N_CORES):
        v = res.results[ci]["vout"]  # (16, O, C)
        outs.append(v.transpose(0, 2, 1))  # -> (16, C, O)
    return np.ascontiguousarray(np.concatenate(outs, axis=0), dtype=np.float32)


# revision 4
# speedup vs baseline: 1.2544x; 1.2544x over previous
"""DigitCaps dynamic-routing kernel for 8x Trainium2 NeuronCores.

Full inputs -> batch-sharded across 8 cores (16 samples/core), W replicated.

Per-core layout:
  u_hat[(r_l,b)=128 partitions, g=256, o=16, c=10]  (bf16 in SBUF)
    where route r = g*8 + r_l   (8 routes per matmul group)
  b_ij / c_ij: (128p, 256g, 10c)   (partition carries (r_l, b))

u_hat build: per group g, one PE matmul:
  lhsT = Xblk[g] (K=64=(r_l,i), M=128=(r_l,b))  block-diagonal x (host-built)
  rhs  = Wt[g]   (K=64=(r_l,i), N=160=(o,c))
  out  = psum (128=(r_l,b), 160=(o,c)) -> drain to SBUF as bf16
  PSUM drains alternate ACT/DVE; iteration-1's s-sum matmuls (c==0.1
  exactly -> s1 = 0.1*sum_r u_hat) are folded into phase 1, accumulating
  into a dedicated PSUM bank as groups drain.

s_j = sum_r c*u_hat: PE matmuls with lhsT = tile(eye(16),(8,8)):
  sums over partitions (r_l) while replicating the (16b, 160) result to all
  128 partitions; 3 groups per matmul accumulated into one PSUM tile.

Iterations 2..3: per 32-group chunk the elementwise work is split between
DVE (28 groups) and GpSimd (4 groups) for the big multiplies/tree, with
GpSimd also taking the b_ij update and c normalization multiply; ACT does
exp. b_ij/c_exp stored bf16 (errors wash out over 2048 routes).
"""

import sys

for p in ("/opt/trn_rl_repo",):
    if p not in sys.path:
        sys.path.insert(0, p)

import numpy as np
import ml_dtypes

import concourse.bass as bass
import concourse.bacc as bacc
import concourse.mybir as mybir
import concourse.tile as tile
from concourse.bass_utils import run_bass_kernel_spmd

# Problem constants (hardcoded per contract)
B_FULL = 128
N_CORES = 8
B = B_FULL // N_CORES  # 16 samples per core
R = 2048
C = 10
O = 16
I = 8
ITERS = 3

RG = 8               # routes per matmul group
G = R // RG          # 256 groups
K = RG * I           # 64 contraction rows per group
CO = C * O           # 160
CH = 32              # groups per routing chunk
NCH = G // CH        # 16 chunks
GP = 4               # groups per chunk handled by GpSimd (rest on DVE)
GD = CH - GP         # DVE groups per chunk
CPAD = 12            # padded capsule dim for 4B alignment of bf16 rows

F32 = mybir.dt.float32
BF16 = mybir.dt.bfloat16

_COMPILED = None  # cache (nc, names) across calls


def _host_prep(x, W):
    """Build per-core DMA-ready arrays. x: (128,2048,8) W: (2048,10,16,8)."""
    x = np.ascontiguousarray(x, dtype=np.float32)
    W = np.ascontiguousarray(W, dtype=np.float32)

    # Wt[g, r_l*8+i, o*10+c] = W[g*8+r_l, c, o, i]
    Wt = W.transpose(0, 3, 2, 1).reshape(G, RG, I, O, C).reshape(G, K, CO)
    # interleave for the build layout: per 8-group sub-chunk, group
    # g0+two*4+gp -> partitions two*64..+63, free slot gp; then pair
    # sub-chunks into 16-group DMA chunks
    Wt8 = (Wt.reshape(G // 8, 2, 4, K, CO).transpose(0, 1, 3, 2, 4)
           .reshape(G // 8, 128, 4, CO))
    Wt16 = (Wt8.reshape(G // 16, 2, 128, 4, CO).transpose(0, 2, 1, 3, 4)
            .reshape(G // 16, 128, 8, CO))

    # Bmask[(r_l,b), (r_l',b')] = 1 if b==b'  -> psum = sum over r_l,
    # replicated across all output partitions
    bmask = np.tile(np.eye(B, dtype=np.float32), (RG, RG))  # (128, 128)
    bmask_bf = bmask.astype(np.float32)

    shards = []
    for ci in range(N_CORES):
        xs = x[ci * B : (ci + 1) * B]  # (16, 2048, 8)
        # xt[g, r_l, i, b] = xs[b, g*8+r_l, i]
        xt = xs.transpose(1, 2, 0).reshape(G, RG, I, B)
        # Block-diagonal lhsT: Xblk[g, r_l*8+i, r_l*16+b] = xt[g, r_l, i, b]
        xblk = np.zeros((G, RG, I, RG, B), dtype=np.float32)
        idx = np.arange(RG)
        xblk[:, idx, :, idx, :] = xt.transpose(1, 0, 2, 3)
        xblk = xblk.reshape(G, K, RG * B)
        xb8 = (xblk.reshape(G // 8, 2, 4, K, RG * B).transpose(0, 1, 3, 2, 4)
               .reshape(G // 8, 128, 4, RG * B))
        xb16 = (xb8.reshape(G // 16, 2, 128, 4, RG * B)
                .transpose(0, 2, 1, 3, 4).reshape(G // 16, 128, 8, RG * B))
        xwt = np.concatenate([xb16, Wt16], axis=3)  # (16, 128, 8, 288)
        shards.append(np.ascontiguousarray(xwt).astype(ml_dtypes.bfloat16))
    return shards, bmask_bf


def _build_kernel():
    nc = bacc.Bacc("TRN2", target_bir_lowering=False, debug=False,
                   num_devices=N_CORES)

    xwt_d = nc.dram_tensor("xwt", [G // 16, 128, 8, 128 + CO], BF16,
                           kind="ExternalInput")
    bmask_d = nc.dram_tensor("bmask", [128, 128], F32, kind="ExternalInput")
    vout_d = nc.dram_tensor("vout", [B, O, C], F32, kind="ExternalOutput")

    with tile.TileContext(nc) as tc:
        with (
            tc.tile_pool(name="persist", bufs=1) as persist,
            tc.tile_pool(name="xw", bufs=3) as xw,
            tc.tile_pool(name="work", bufs=6) as work,
            tc.tile_pool(name="psum", bufs=6, space="PSUM") as psum,
            tc.tile_pool(name="spsum", bufs=2, space="PSUM") as spsum,
        ):
            uhat = persist.tile([128, G, O, C], BF16)      # 80 KiB/part
            bij = persist.tile([128, G, C], BF16)          # 5 KiB
            cexp = persist.tile([128, G, C], BF16)         # 5 KiB
            cbf = persist.tile([128, G, CPAD], BF16)       # 6 KiB
            bmask = persist.tile([128, 128], F32)
            bmask_b = persist.tile([128, 128], BF16)
            v_bf = persist.tile([128, O, C], BF16)
            s_sb = persist.tile([128, O, C], F32)
            sq = persist.tile([128, C], F32)
            sq2 = persist.tile([128, C], F32)
            zsum = persist.tile([128, G], F32)
            zrec = persist.tile([128, G], BF16)
            eps_t = persist.tile([128, 1], F32)
            nc.gpsimd.memset(eps_t[:], 1e-8)

            nc.sync.dma_start(bmask[:], bmask_d[:])
            nc.vector.tensor_copy(bmask_b[:], bmask[:])

            # ---------- Phase 1: u_hat build + iter-1 s accumulation ----
            # Group pairs (g0+j, g0+4+j) run concurrently in PE row-groups
            # 0-63 / 64-127 (K=64 each); their outputs go to different PSUM
            # banks so the row-tiles don't serialize on the bank tracker.
            # One 576KiB DMA per 16-group chunk; within: 2 sub-chunks of 8
            # groups. Group g0+two*4+gp -> partitions two*64..+63, free slot
            # (sub*4+gp); xb = slot[..., 0:128], wt = slot[..., 128:288].
            # As each 16-group chunk is drained, its contribution to
            # s1 = 0.1*sum_r u_hat accumulates into the sp PSUM tile (the
            # iteration-1 c_ij are exactly 0.1).
            sp = spsum.tile([128, 3, O, C], F32, tag="sp")
            for dc in range(G // 16):
                xwt_t = xw.tile([128, 8, 128 + CO], BF16, tag="xwt")
                eng = nc.sync if dc % 2 == 0 else nc.scalar
                eng.dma_start(xwt_t[:], xwt_d[dc])
                for sub in range(2):
                    g0 = dc * 16 + sub * 8
                    pst = [psum.tile([128, 2, O, C], F32, tag="ps",
                                     name=f"ps{g0}_{t}") for t in range(4)]
                    for j in range(4):
                        for two in range(2):
                            g = g0 + two * 4 + j
                            lo = (g - g0) % 8
                            sl = xwt_t[two * 64 : two * 64 + 64, sub * 4 + j]
                            nc.tensor.matmul(
                                pst[lo // 2][:, lo & 1],
                                lhsT=sl[:, 0:128],
                                rhs=sl[:, 128 : 128 + CO],
                                start=True, stop=True)
                    for t in range(4):
                        if t % 2 == 0:
                            nc.scalar.copy(
                                uhat[:, g0 + 2 * t : g0 + 2 * t + 2], pst[t][:])
                        else:
                            nc.vector.tensor_copy(
                                uhat[:, g0 + 2 * t : g0 + 2 * t + 2], pst[t][:])
                # iter-1 s-sum over this chunk's 16 groups: 5 triples + 1
                gc = dc * 16
                for m in range(5):
                    nc.tensor.matmul(
                        sp[:], lhsT=bmask_b[:],
                        rhs=uhat[:, gc + 3 * m : gc + 3 * m + 3].rearrange(
                            "p g o c -> p (g o c)"),
                        start=(dc == 0 and m == 0), stop=False)
                nc.tensor.matmul(
                    sp[:, 0], lhsT=bmask_b[:],
                    rhs=uhat[:, gc + 15].rearrange("p o c -> p (o c)"),
                    start=False, stop=(dc == G // 16 - 1))

            # ---------- helpers ----------
            def s_combine(sp, scale):
                # s_sb = (sp0 + sp1 + sp2) * scale; only one PSUM read per op
                nc.scalar.copy(s_sb[:], sp[:, 0])
                nc.vector.tensor_add(s_sb[:], s_sb[:], sp[:, 1])
                nc.vector.tensor_add(s_sb[:], s_sb[:], sp[:, 2])
                if scale != 1.0:
                    nc.scalar.mul(s_sb[:], s_sb[:], scale)

            # ---------- iteration 1 (s already accumulated) ----------
            s_combine(sp, 0.1)
            _squash(nc, work, s_sb, sq, sq2, v_bf, eps_t)

            # ---------- iterations 2..3 ----------
            # All elementwise work stays on DVE (GpSimd contends with DVE
            # for the shared SBUF port pair and is ~3x slower per element,
            # so offloading to it is a net loss). b_ij/c_exp/z_rec are bf16
            # so the small softmax ops run in the 2x DVE perf mode; the
            # errors are per-route and wash out over the 2048-route s-sum.
            for it in range(1, ITERS):
                sp = spsum.tile([128, 3, O, C], F32, tag="sp",
                                name=f"sp_{it}")

                def agree_mult(ch):
                    g0 = ch * CH
                    pa = work.tile([128, CH, O, C], BF16, tag="prod",
                                   name=f"pa{it}_{ch}")
                    nc.vector.tensor_mul(
                        pa[:], uhat[:, g0 : g0 + CH],
                        v_bf[:].unsqueeze(1).broadcast_to((128, CH, O, C)))
                    return pa

                pa = agree_mult(0)
                for ch in range(NCH):
                    g0 = ch * CH
                    sl = slice(g0, g0 + CH)
                    nc.vector.tensor_add(pa[:, :, 0:8], pa[:, :, 0:8],
                                         pa[:, :, 8:16])
                    nc.vector.tensor_add(pa[:, :, 0:4], pa[:, :, 0:4],
                                         pa[:, :, 4:8])
                    nc.vector.tensor_add(pa[:, :, 0:2], pa[:, :, 0:2],
                                         pa[:, :, 2:4])
                    if it == 1:
                        nc.vector.tensor_add(bij[:, sl],
                                             pa[:, :, 0], pa[:, :, 1])
                    else:
                        nc.vector.tensor_add(pa[:, :, 0], pa[:, :, 0],
                                             pa[:, :, 1])
                        nc.vector.tensor_add(bij[:, sl],
                                             bij[:, sl], pa[:, :, 0])
                    # chunk-local softmax over c (exp on ACT)
                    nc.scalar.activation(cexp[:, sl], bij[:, sl],
                                         mybir.ActivationFunctionType.Exp)
                    pa_next = agree_mult(ch + 1) if ch + 1 < NCH else None
                    nc.vector.reduce_sum(zsum[:, sl], cexp[:, sl],
                                         axis=mybir.AxisListType.X)
                    with nc.allow_low_precision(
                            reason="per-route softmax scale; bf16 error is "
                            "a common factor per route and washes out over "
                            "the 2048-route sum"):
                        nc.vector.reciprocal(zrec[:, sl], zsum[:, sl])
                    nc.vector.tensor_mul(
                        cbf[:, sl, 0:C], cexp[:, sl],
                        zrec[:, sl].unsqueeze(2).broadcast_to((128, CH, C)))
                    # s products + PE group-triple sums
                    prods = work.tile([128, CH, O, C], BF16, tag="prod",
                                      name=f"psx{it}_{ch}")
                    nc.vector.tensor_mul(
                        prods[:], uhat[:, sl],
                        cbf[:, sl, 0:C].unsqueeze(2)
                        .broadcast_to((128, CH, O, C)))
                    # 10 triples + one pair per 32-group chunk
                    for j in range(10):
                        nc.tensor.matmul(
                            sp[:], lhsT=bmask_b[:],
                            rhs=prods[:, 3 * j : 3 * j + 3].rearrange(
                                "p g o c -> p (g o c)"),
                            start=(ch == 0 and j == 0), stop=False)
                    nc.tensor.matmul(
                        sp[:, 0:2], lhsT=bmask_b[:],
                        rhs=prods[:, 30:32].rearrange("p g o c -> p (g o c)"),
                        start=False, stop=(ch == NCH - 1))
                    pa = pa_next
                s_combine(sp, 1.0)
                _squash(nc, work, s_sb, sq, sq2, v_bf, eps_t)

            # ---------- output ----------
            vfin = work.tile([128, O, C], F32, tag="vfin")
            nc.vector.tensor_mul(
                vfin[:], s_sb[:],
                sq[:].unsqueeze(1).broadcast_to((128, O, C)))
            nc.sync.dma_start(vout_d[:], vfin[0:B])

    nc.compile()
    return nc


def _squash(nc, work, s_sb, sq, sq2, v16, eps_t):
    """v = s * (|s|^2/(1+|s|^2)) / sqrt(|s|^2 + 1e-8), per (b, c).

    Runs on B=16 partitions; leaves the scale factor in `sq`;
    v16 = s * scale (bf16). s_sb layout (B, O, C).
    """
    P = s_sb.shape[0]
    ssq = work.tile([P, O, C], F32, tag="ssq")
    nc.vector.tensor_mul(ssq[:], s_sb[:], s_sb[:])
    nc.vector.reduce_sum(sq[:], ssq[:].rearrange("p o c -> p c o"),
                         axis=mybir.AxisListType.X)
    # sq2 = (1+n)*sqrt(n+1e-8);  sq = n / sq2
    nc.scalar.activation(sq2[:], sq[:], mybir.ActivationFunctionType.Sqrt,
                         bias=eps_t[0:P])
    nc.vector.scalar_tensor_tensor(
        sq2[:], sq[:], 1.0, sq2[:],
        op0=mybir.AluOpType.add, op1=mybir.AluOpType.mult)
    nc.vector.reciprocal(sq2[:], sq2[:])
    nc.vector.tensor_mul(sq[:], sq[:], sq2[:])
    nc.vector.tensor_mul(
        v16[:], s_sb[:], sq[:].unsqueeze(1).broadcast_to((P, O, C)))


def kernel(x, W):
    global _COMPILED
    xshards, bmask = _host_prep(x, W)
    if _COMPILED is None:
        _COMPILED = _build_kernel()
    nc = _COMPILED
    in_maps = [
        {"xwt": xs, "bmask": bmask} for xs in xshards
    ]
    res = run_bass_kernel_spmd(nc, in_maps, list(range(N_CORES)))
    outs = []
    for ci in range(N_CORES):
        v = res.results[ci]["vout"]  # (16, O, C)
        outs.append(v.transpose(0, 2, 1))  # -> (16, C, O)
    return np.ascontiguousarray(np.concatenate(outs, axis=0), dtype=np.float32)


# revision 17
# speedup vs baseline: 1.4428x; 1.1502x over previous
"""DigitCaps dynamic-routing kernel for 8x Trainium2 NeuronCores.

Full inputs -> batch-sharded across 8 cores (16 samples/core), W replicated.

Per-core layout:
  u_hat[(r_l,b)=128 partitions, g=256, o=16, c=10]  (bf16 in SBUF)
    where route r = g*8 + r_l   (8 routes per matmul group)
  b_ij / c_ij: (128p, 256g, 10c)   (partition carries (r_l, b))

u_hat build: per group g, one PE matmul:
  lhsT = Xblk[g] (K=64=(r_l,i), M=128=(r_l,b))  block-diagonal x (host-built)
  rhs  = Wt[g]   (K=64=(r_l,i), N=160=(o,c))
  out  = psum (128=(r_l,b), 160=(o,c)) -> drain to SBUF as bf16
  PSUM drains alternate ACT/DVE; iteration-1's s-sum matmuls (c==0.1
  exactly -> s1 = 0.1*sum_r u_hat) are folded into phase 1, accumulating
  into a dedicated PSUM bank as groups drain.

s_j = sum_r c*u_hat: PE matmuls with lhsT = tile(eye(16),(8,8)):
  sums over partitions (r_l) while replicating the (16b, 160) result to all
  128 partitions; 3 groups per matmul accumulated into one PSUM tile.

Iterations 2..3: per 32-group chunk the elementwise work is split between
DVE (28 groups) and GpSimd (4 groups) for the big multiplies/tree, with
GpSimd also taking the b_ij update and c normalization multiply; ACT does
exp. b_ij/c_exp stored bf16 (errors wash out over 2048 routes).
"""

import sys

for p in ("/opt/trn_rl_repo",):
    if p not in sys.path:
        sys.path.insert(0, p)

import numpy as np
import ml_dtypes

import concourse.bass as bass
import concourse.bacc as bacc
import concourse.mybir as mybir
import concourse.tile as tile
from concourse.bass_utils import run_bass_kernel_spmd

# Problem constants (hardcoded per contract)
B_FULL = 128
N_CORES = 8
B = B_FULL // N_CORES  # 16 samples per core
R = 2048
C = 10
O = 16
I = 8
ITERS = 3

RG = 8               # routes per matmul group
G = R // RG          # 256 groups
K = RG * I           # 64 contraction rows per group
CO = C * O           # 160
CH = 32              # groups per routing chunk
NCH = G // CH        # 16 chunks
GP = 4               # groups per chunk handled by GpSimd (rest on DVE)
GD = CH - GP         # DVE groups per chunk
CPAD = 12            # padded capsule dim for 4B alignment of bf16 rows

F32 = mybir.dt.float32
BF16 = mybir.dt.bfloat16

_COMPILED = None  # cache (nc, names) across calls


def _host_prep(x, W):
    """Build per-core DMA-ready arrays. x: (128,2048,8) W: (2048,10,16,8).

    Returns (xcs, wt, bmask): xcs[ci] is the compact per-core x
    [p=(two,r_l,i)=128, gidx=(dc,sub,j)=128, b=16] (the block-diagonal
    lhsT is expanded on-chip), wt is the replicated weight layout
    [dc, 128, slot=8, 160], bmask the r_l-summing replication mask.
    """
    x = np.ascontiguousarray(x, dtype=np.float32)
    W = np.ascontiguousarray(W, dtype=np.float32)

    # Wt[g, r_l*8+i, o*10+c] = W[g*8+r_l, c, o, i]
    Wt = W.transpose(0, 3, 2, 1).reshape(G, RG, I, O, C).reshape(G, K, CO)
    # interleave for the build layout: per 8-group sub-chunk, group
    # g0+two*4+gp -> partitions two*64..+63, free slot gp; then pair
    # sub-chunks into 16-group DMA chunks
    Wt8 = (Wt.reshape(G // 8, 2, 4, K, CO).transpose(0, 1, 3, 2, 4)
           .reshape(G // 8, 128, 4, CO))
    Wt16 = (Wt8.reshape(G // 16, 2, 128, 4, CO).transpose(0, 2, 1, 3, 4)
            .reshape(G // 16, 128, 8, CO))

    # Bmask[(r_l,b), (r_l',b')] = 1 if b==b'  -> psum = sum over r_l,
    # replicated across all output partitions
    bmask = np.tile(np.eye(B, dtype=np.float32), (RG, RG))  # (128, 128)
    bmask_bf = bmask.astype(np.float32)

    shards = []
    xcs = []
    for ci in range(N_CORES):
        xs = x[ci * B : (ci + 1) * B]  # (16, 2048, 8)
        # xt[g, r_l, i, b] = xs[b, g*8+r_l, i]
        xt = xs.transpose(1, 2, 0).reshape(G, RG, I, B)
        # compact x for the s1 accumulation:
        # g = dc*16 + sub*8 + two*4 + j -> p=(two,r_l,i), gidx=(dc,sub,j)
        xc = (xt.reshape(16, 2, 2, 4, RG, I, B)
              .transpose(2, 4, 5, 0, 1, 3, 6).reshape(128, 128, B))
        xcs.append(np.ascontiguousarray(xc).astype(ml_dtypes.bfloat16))
        # Block-diagonal lhsT: Xblk[g, r_l*8+i, r_l*16+b] = xt[g, r_l, i, b]
        xblk = np.zeros((G, RG, I, RG, B), dtype=np.float32)
        idx = np.arange(RG)
        xblk[:, idx, :, idx, :] = xt.transpose(1, 0, 2, 3)
        xblk = xblk.reshape(G, K, RG * B)
        xb8 = (xblk.reshape(G // 8, 2, 4, K, RG * B).transpose(0, 1, 3, 2, 4)
               .reshape(G // 8, 128, 4, RG * B))
        xb16 = (xb8.reshape(G // 16, 2, 128, 4, RG * B)
                .transpose(0, 2, 1, 3, 4).reshape(G // 16, 128, 8, RG * B))
        xwt = np.concatenate([xb16, Wt16], axis=3)  # (16, 128, 8, 288)
        shards.append(np.ascontiguousarray(xwt).astype(ml_dtypes.bfloat16))
    return shards, xcs, bmask_bf


def _build_kernel():
    nc = bacc.Bacc("TRN2", target_bir_lowering=False, debug=False,
                   num_devices=N_CORES)

    xwt_d = nc.dram_tensor("xwt", [G // 16, 128, 8, 128 + CO], BF16,
                           kind="ExternalInput")
    xc_d = nc.dram_tensor("xc", [128, 128, B], BF16, kind="ExternalInput")
    bmask_d = nc.dram_tensor("bmask", [128, 128], F32, kind="ExternalInput")
    vout_d = nc.dram_tensor("vout", [B, O, C], F32, kind="ExternalOutput")

    with tile.TileContext(nc) as tc:
        with (
            tc.tile_pool(name="persist", bufs=1) as persist,
            tc.tile_pool(name="xw", bufs=3) as xw,
            tc.tile_pool(name="work", bufs=6) as work,
            tc.tile_pool(name="psum", bufs=6, space="PSUM") as psum,
            tc.tile_pool(name="spsum", bufs=2, space="PSUM") as spsum,
        ):
            uhat = persist.tile([128, G, O, C], BF16)      # 80 KiB/part
            xc_sb = persist.tile([128, 128, B], BF16)      # 4 KiB/part
            bij = persist.tile([128, G, C], BF16)          # 5 KiB
            cexp = persist.tile([128, G, C], BF16)         # 5 KiB
            cbf = persist.tile([128, G, CPAD], BF16)       # 6 KiB
            bmask = persist.tile([128, 128], F32)
            bmask_b = persist.tile([128, 128], BF16)
            v_bf = persist.tile([128, O, C], BF16)
            s_sb = persist.tile([128, O, C], F32)
            sq = persist.tile([128, C], F32)
            sq2 = persist.tile([128, C], F32)
            zsum = persist.tile([128, G], F32)
            zrec = persist.tile([128, G], BF16)
            eps_t = persist.tile([128, 1], F32)
            nc.gpsimd.memset(eps_t[:], 1e-8)

            nc.sync.dma_start(xc_sb[:], xc_d[:])
            nc.sync.dma_start(bmask[:], bmask_d[:])
            nc.vector.tensor_copy(bmask_b[:], bmask[:])

            # ---------- Phase 1: u_hat build + iter-1 s accumulation ----
            # Group pairs (g0+j, g0+4+j) run concurrently in PE row-groups
            # 0-63 / 64-127 (K=64 each); their outputs go to different PSUM
            # banks so the row-tiles don't serialize on the bank tracker.
            # PSUM drains alternate ACT/DVE. Iteration-1's s1 = 0.1*sum_r
            # u_hat is accumulated directly from the inputs: one K=128
            # matmul per slot (lhsT = xc slot (128,16), rhs = wt slot
            # (128,160)) summing both groups' routes, accumulated over all
            # 128 slots into one PSUM tile -- no dependency on the drains.
            sp1 = spsum.tile([128, O, C], F32, tag="sp", name="s1acc")
            for dc in range(G // 16):
                xwt_t = xw.tile([128, 8, 128 + CO], BF16, tag="xwt")
                eng = nc.sync if dc % 2 == 0 else nc.scalar
                eng.dma_start(xwt_t[:], xwt_d[dc])
                for sub in range(2):
                    g0 = dc * 16 + sub * 8
                    pst = [psum.tile([128, 2, O, C], F32, tag="ps",
                                     name=f"ps{g0}_{t}") for t in range(4)]
                    for j in range(4):
                        for two in range(2):
                            g = g0 + two * 4 + j
                            lo = (g - g0) % 8
                            sl = xwt_t[two * 64 : two * 64 + 64, sub * 4 + j]
                            nc.tensor.matmul(
                                pst[lo // 2][:, lo & 1],
                                lhsT=sl[:, 0:128],
                                rhs=sl[:, 128 : 128 + CO],
                                start=True, stop=True)
                    for t in range(4):
                        if t % 2 == 0:
                            nc.scalar.copy(
                                uhat[:, g0 + 2 * t : g0 + 2 * t + 2], pst[t][:])
                        else:
                            nc.vector.tensor_copy(
                                uhat[:, g0 + 2 * t : g0 + 2 * t + 2], pst[t][:])
                    for j in range(4):
                        gidx = dc * 8 + sub * 4 + j
                        nc.tensor.matmul(
                            sp1[0:B], lhsT=xc_sb[:, gidx],
                            rhs=xwt_t[:, sub * 4 + j, 128 : 128 + CO],
                            start=(dc == 0 and sub == 0 and j == 0),
                            stop=(dc == G // 16 - 1 and sub == 1 and j == 3))

            # ---------- helpers ----------
            def s_combine(sp, scale):
                # s_sb = (sp0 + sp1 + sp2) * scale; only one PSUM read per op
                nc.scalar.copy(s_sb[:], sp[:, 0])
                nc.vector.tensor_add(s_sb[:], s_sb[:], sp[:, 1])
                nc.vector.tensor_add(s_sb[:], s_sb[:], sp[:, 2])
                if scale != 1.0:
                    nc.scalar.mul(s_sb[:], s_sb[:], scale)

            # ---------- iteration 1 ----------
            # Squash on the 16 real (b) partitions of the s1 accumulator,
            # then replicate v to all (r_l,b) partitions via PE (bmask rows
            # 0:16 are exactly the replication mask; bf16 exact for 0/1).
            nc.scalar.copy(s_sb[0:B], sp1[0:B])
            nc.scalar.mul(s_sb[0:B], s_sb[0:B], 0.1)
            _squash(nc, work, s_sb[0:B], sq[0:B], sq2[0:B], v_bf[0:B],
                    eps_t)
            rep = spsum.tile([128, O, C], F32, tag="sp", name="srep")
            nc.tensor.matmul(rep[:], lhsT=bmask_b[0:B], rhs=v_bf[0:B],
                             start=True, stop=True)
            nc.vector.tensor_copy(v_bf[:], rep[:])

            # ---------- iterations 2..3 ----------
            # All elementwise work stays on DVE (GpSimd contends with DVE
            # for the shared SBUF port pair and is ~3x slower per element,
            # so offloading to it is a net loss). b_ij/c_exp/z_rec are bf16
            # so the small softmax ops run in the 2x DVE perf mode; the
            # errors are per-route and wash out over the 2048-route s-sum.
            for it in range(1, ITERS):
                sp = spsum.tile([128, 3, O, C], F32, tag="sp",
                                name=f"sp_{it}")

                def agree_mult(ch):
                    g0 = ch * CH
                    pa = work.tile([128, CH, O, C], BF16, tag="prod",
                                   name=f"pa{it}_{ch}")
                    nc.vector.tensor_mul(
                        pa[:], uhat[:, g0 : g0 + CH],
                        v_bf[:].unsqueeze(1).broadcast_to((128, CH, O, C)))
                    return pa

                pa = agree_mult(0)
                for ch in range(NCH):
                    g0 = ch * CH
                    sl = slice(g0, g0 + CH)
                    nc.vector.tensor_add(pa[:, :, 0:8], pa[:, :, 0:8],
                                         pa[:, :, 8:16])
                    nc.vector.tensor_add(pa[:, :, 0:4], pa[:, :, 0:4],
                                         pa[:, :, 4:8])
                    nc.vector.tensor_add(pa[:, :, 0:2], pa[:, :, 0:2],
                                         pa[:, :, 2:4])
                    if it == 1:
                        nc.vector.tensor_add(bij[:, sl],
                                             pa[:, :, 0], pa[:, :, 1])
                    else:
                        nc.vector.tensor_add(pa[:, :, 0], pa[:, :, 0],
                                             pa[:, :, 1])
                        nc.vector.tensor_add(bij[:, sl],
                                             bij[:, sl], pa[:, :, 0])
                    # chunk-local softmax over c (exp on ACT)
                    nc.scalar.activation(cexp[:, sl], bij[:, sl],
                                         mybir.ActivationFunctionType.Exp)
                    pa_next = agree_mult(ch + 1) if ch + 1 < NCH else None
                    nc.vector.reduce_sum(zsum[:, sl], cexp[:, sl],
                                         axis=mybir.AxisListType.X)
                    with nc.allow_low_precision(
                            reason="per-route softmax scale; bf16 error is "
                            "a common factor per route and washes out over "
                            "the 2048-route sum"):
                        nc.vector.reciprocal(zrec[:, sl], zsum[:, sl])
                    nc.vector.tensor_mul(
                        cbf[:, sl, 0:C], cexp[:, sl],
                        zrec[:, sl].unsqueeze(2).broadcast_to((128, CH, C)))
                    # s products + PE group-triple sums
                    prods = work.tile([128, CH, O, C], BF16, tag="prod",
                                      name=f"psx{it}_{ch}")
                    nc.vector.tensor_mul(
                        prods[:], uhat[:, sl],
                        cbf[:, sl, 0:C].unsqueeze(2)
                        .broadcast_to((128, CH, O, C)))
                    # 10 triples + one pair per 32-group chunk
                    for j in range(10):
                        nc.tensor.matmul(
                            sp[:], lhsT=bmask_b[:],
                            rhs=prods[:, 3 * j : 3 * j + 3].rearrange(
                                "p g o c -> p (g o c)"),
                            start=(ch == 0 and j == 0), stop=False)
                    nc.tensor.matmul(
                        sp[:, 0:2], lhsT=bmask_b[:],
                        rhs=prods[:, 30:32].rearrange("p g o c -> p (g o c)"),
                        start=False, stop=(ch == NCH - 1))
                    pa = pa_next
                s_combine(sp, 1.0)
                _squash(nc, work, s_sb, sq, sq2, v_bf, eps_t)

            # ---------- output ----------
            vfin = work.tile([128, O, C], F32, tag="vfin")
            nc.vector.tensor_mul(
                vfin[:], s_sb[:],
                sq[:].unsqueeze(1).broadcast_to((128, O, C)))
            nc.sync.dma_start(vout_d[:], vfin[0:B])

    nc.compile()
    return nc


def _squash(nc, work, s_sb, sq, sq2, v16, eps_t):
    """v = s * (|s|^2/(1+|s|^2)) / sqrt(|s|^2 + 1e-8), per (b, c).

    Runs on B=16 partitions; leaves the scale factor in `sq`;
    v16 = s * scale (bf16). s_sb layout (B, O, C).
    """
    P = s_sb.shape[0]
    ssq = work.tile([P, O, C], F32, tag="ssq")
    nc.vector.tensor_mul(ssq[:], s_sb[:], s_sb[:])
    nc.vector.reduce_sum(sq[:], ssq[:].rearrange("p o c -> p c o"),
                         axis=mybir.AxisListType.X)
    # sq2 = (1+n)*sqrt(n+1e-8);  sq = n / sq2
    nc.scalar.activation(sq2[:], sq[:], mybir.ActivationFunctionType.Sqrt,
                         bias=eps_t[0:P])
    nc.vector.scalar_tensor_tensor(
        sq2[:], sq[:], 1.0, sq2[:],
        op0=mybir.AluOpType.add, op1=mybir.AluOpType.mult)
    nc.vector.reciprocal(sq2[:], sq2[:])
    nc.vector.tensor_mul(sq[:], sq[:], sq2[:])
    nc.vector.tensor_mul(
        v16[:], s_sb[:], sq[:].unsqueeze(1).broadcast_to((P, O, C)))


def kernel(x, W):
    global _COMPILED
    shards, xcs, bmask = _host_prep(x, W)
    if _COMPILED is None:
        _COMPILED = _build_kernel()
    nc = _COMPILED
    in_maps = [
        {"xwt": xs, "xc": xc, "bmask": bmask}
        for xs, xc in zip(shards, xcs)
    ]
    res = run_bass_kernel_spmd(nc, in_maps, list(range(N_CORES)))
    outs = []
    for ci in range(N_CORES):
        v = res.results[ci]["vout"]  # (16, O, C)
        outs.append(v.transpose(0, 2, 1))  # -> (16, C, O)
    return np.ascontiguousarray(np.concatenate(outs, axis=0), dtype=np.float32)


# revision 23
# speedup vs baseline: 1.5149x; 1.0500x over previous
"""DigitCaps dynamic-routing kernel for 8x Trainium2 NeuronCores.

Full inputs -> batch-sharded across 8 cores (16 samples/core), W replicated.

Per-core layout:
  u_hat[(r_l,b)=128 partitions, g=256, o=16, c=10]  (bf16 in SBUF)
    where route r = g*8 + r_l   (8 routes per matmul group)
  b_ij / c_ij: (128p, 256g, 10c)   (partition carries (r_l, b))

u_hat build: per group g, one PE matmul:
  lhsT = Xblk[g] (K=64=(r_l,i), M=128=(r_l,b))  block-diagonal x (host-built)
  rhs  = Wt[g]   (K=64=(r_l,i), N=160=(o,c))
  out  = psum (128=(r_l,b), 160=(o,c)) -> drain to SBUF as bf16
  PSUM drains alternate ACT/DVE; iteration-1's s-sum matmuls (c==0.1
  exactly -> s1 = 0.1*sum_r u_hat) are folded into phase 1, accumulating
  into a dedicated PSUM bank as groups drain.

s_j = sum_r c*u_hat: PE matmuls with lhsT = tile(eye(16),(8,8)):
  sums over partitions (r_l) while replicating the (16b, 160) result to all
  128 partitions; 3 groups per matmul accumulated into one PSUM tile.

Iterations 2..3: per 32-group chunk the elementwise work is split between
DVE (28 groups) and GpSimd (4 groups) for the big multiplies/tree, with
GpSimd also taking the b_ij update and c normalization multiply; ACT does
exp. b_ij/c_exp stored bf16 (errors wash out over 2048 routes).
"""

import sys

for p in ("/opt/trn_rl_repo",):
    if p not in sys.path:
        sys.path.insert(0, p)

import numpy as np
import ml_dtypes

import concourse.bass as bass
import concourse.bacc as bacc
import concourse.mybir as mybir
import concourse.tile as tile
from concourse.bass_utils import run_bass_kernel_spmd

# Problem constants (hardcoded per contract)
B_FULL = 128
N_CORES = 8
B = B_FULL // N_CORES  # 16 samples per core
R = 2048
C = 10
O = 16
I = 8
ITERS = 3

RG = 8               # routes per matmul group
G = R // RG          # 256 groups
K = RG * I           # 64 contraction rows per group
CO = C * O           # 160
CH = 32              # groups per routing chunk
NCH = G // CH        # 16 chunks
GP = 4               # groups per chunk handled by GpSimd (rest on DVE)
GD = CH - GP         # DVE groups per chunk
CPAD = 12            # padded capsule dim for 4B alignment of bf16 rows

F32 = mybir.dt.float32
BF16 = mybir.dt.bfloat16

_COMPILED = None  # cache (nc, names) across calls


def _host_prep(x, W):
    """Build per-core DMA-ready arrays. x: (128,2048,8) W: (2048,10,16,8).

    Returns (xcs, wt, bmask): xcs[ci] is the compact per-core x
    [p=(two,r_l,i)=128, gidx=(dc,sub,j)=128, b=16] (the block-diagonal
    lhsT is expanded on-chip), wt is the replicated weight layout
    [dc, 128, slot=8, 160], bmask the r_l-summing replication mask.
    """
    x = np.ascontiguousarray(x, dtype=np.float32)
    W = np.ascontiguousarray(W, dtype=np.float32)

    # Wt[g, r_l*8+i, o*10+c] = W[g*8+r_l, c, o, i]
    Wt = W.transpose(0, 3, 2, 1).reshape(G, RG, I, O, C).reshape(G, K, CO)
    # interleave for the build layout: per 8-group sub-chunk, group
    # g0+two*4+gp -> partitions two*64..+63, free slot gp; then pair
    # sub-chunks into 16-group DMA chunks
    Wt8 = (Wt.reshape(G // 8, 2, 4, K, CO).transpose(0, 1, 3, 2, 4)
           .reshape(G // 8, 128, 4, CO))
    Wt16 = (Wt8.reshape(G // 16, 2, 128, 4, CO).transpose(0, 2, 1, 3, 4)
            .reshape(G // 16, 128, 8, CO))

    # Bmask[(r_l,b), (r_l',b')] = 1 if b==b'  -> psum = sum over r_l,
    # replicated across all output partitions
    bmask = np.tile(np.eye(B, dtype=np.float32), (RG, RG))  # (128, 128)
    bmask_bf = bmask.astype(np.float32)

    shards = []
    xcs = []
    for ci in range(N_CORES):
        xs = x[ci * B : (ci + 1) * B]  # (16, 2048, 8)
        # xt[g, r_l, i, b] = xs[b, g*8+r_l, i]
        xt = xs.transpose(1, 2, 0).reshape(G, RG, I, B)
        # compact x for the s1 accumulation:
        # g = dc*16 + sub*8 + two*4 + j -> p=(two,r_l,i), gidx=(dc,sub,j)
        xc = (xt.reshape(16, 2, 2, 4, RG, I, B)
              .transpose(2, 4, 5, 0, 1, 3, 6).reshape(128, 128, B))
        xcs.append(np.ascontiguousarray(xc).astype(ml_dtypes.bfloat16))
        # Block-diagonal lhsT: Xblk[g, r_l*8+i, r_l*16+b] = xt[g, r_l, i, b]
        xblk = np.zeros((G, RG, I, RG, B), dtype=np.float32)
        idx = np.arange(RG)
        xblk[:, idx, :, idx, :] = xt.transpose(1, 0, 2, 3)
        xblk = xblk.reshape(G, K, RG * B)
        xb8 = (xblk.reshape(G // 8, 2, 4, K, RG * B).transpose(0, 1, 3, 2, 4)
               .reshape(G // 8, 128, 4, RG * B))
        xb16 = (xb8.reshape(G // 16, 2, 128, 4, RG * B)
                .transpose(0, 2, 1, 3, 4).reshape(G // 16, 128, 8, RG * B))
        xwt = np.concatenate([xb16, Wt16], axis=3)  # (16, 128, 8, 288)
        # pair 16-group chunks into 32-group DMA chunks: (8, 128, 16, 288)
        xwt2 = (xwt.reshape(8, 2, 128, 8, 128 + CO).transpose(0, 2, 1, 3, 4)
                .reshape(8, 128, 16, 128 + CO))
        shards.append(np.ascontiguousarray(xwt2).astype(ml_dtypes.bfloat16))
    return shards, xcs, bmask_bf


def _build_kernel():
    nc = bacc.Bacc("TRN2", target_bir_lowering=False, debug=False,
                   num_devices=N_CORES)

    xwt_d = nc.dram_tensor("xwt", [G // 32, 128, 16, 128 + CO], BF16,
                           kind="ExternalInput")
    xc_d = nc.dram_tensor("xc", [128, 128, B], BF16, kind="ExternalInput")
    bmask_d = nc.dram_tensor("bmask", [128, 128], F32, kind="ExternalInput")
    vout_d = nc.dram_tensor("vout", [B, O, C], F32, kind="ExternalOutput")

    with tile.TileContext(nc) as tc:
        with (
            tc.tile_pool(name="persist", bufs=1) as persist,
            tc.tile_pool(name="xw", bufs=3) as xw,
            tc.tile_pool(name="work", bufs=6) as work,
            tc.tile_pool(name="psum", bufs=6, space="PSUM") as psum,
            tc.tile_pool(name="spsum", bufs=2, space="PSUM") as spsum,
        ):
            uhat = persist.tile([128, G, O, C], BF16)      # 80 KiB/part
            xc_sb = persist.tile([128, 128, B], BF16)      # 4 KiB/part
            bij = persist.tile([128, G, C], BF16)          # 5 KiB
            cexp = persist.tile([128, G, C], BF16)         # 5 KiB
            cbf = persist.tile([128, G, CPAD], BF16)       # 6 KiB
            bmask = persist.tile([128, 128], F32)
            bmask_b = persist.tile([128, 128], BF16)
            v_bf = persist.tile([128, O, C], BF16)
            s_sb = persist.tile([128, O, C], F32)
            sq = persist.tile([128, C], F32)
            sq2 = persist.tile([128, C], F32)
            zsum = persist.tile([128, G], F32)
            zrec = persist.tile([128, G], BF16)
            eps_t = persist.tile([128, 1], F32)
            nc.gpsimd.memset(eps_t[:], 1e-8)

            nc.sync.dma_start(xc_sb[:], xc_d[:])
            nc.sync.dma_start(bmask[:], bmask_d[:])
            nc.vector.tensor_copy(bmask_b[:], bmask[:])

            # ---------- Phase 1: u_hat build + iter-1 s accumulation ----
            # Group pairs (g0+j, g0+4+j) run concurrently in PE row-groups
            # 0-63 / 64-127 (K=64 each); their outputs go to different PSUM
            # banks so the row-tiles don't serialize on the bank tracker.
            # PSUM drains alternate ACT/DVE. Iteration-1's s1 = 0.1*sum_r
            # u_hat is accumulated directly from the inputs: one K=128
            # matmul per slot (lhsT = xc slot (128,16), rhs = wt slot
            # (128,160)) summing both groups' routes, accumulated over all
            # 128 slots into one PSUM tile -- no dependency on the drains.
            sp1 = spsum.tile([128, O, C], F32, tag="sp", name="s1acc")
            for dc2 in range(G // 32):
                xwt_t = xw.tile([128, 16, 128 + CO], BF16, tag="xwt")
                eng = nc.sync if dc2 % 2 == 0 else nc.scalar
                eng.dma_start(xwt_t[:], xwt_d[dc2])
                for sub4 in range(4):
                    g0 = dc2 * 32 + sub4 * 8
                    slot0 = sub4 * 4
                    # 3 PSUM tiles per 8-group sub-chunk: groups [0:3],
                    # [3:6], [6:8]; concurrent row-group pairs (j, j+4)
                    # always land in different tiles.
                    pst = [psum.tile([128, 3, O, C], F32, tag="ps",
                                     name=f"ps{g0}_{t}") for t in range(3)]
                    for j in range(4):
                        for two in range(2):
                            lo = two * 4 + j
                            sl = xwt_t[two * 64 : two * 64 + 64, slot0 + j]
                            nc.tensor.matmul(
                                pst[lo // 3][:, lo % 3],
                                lhsT=sl[:, 0:128],
                                rhs=sl[:, 128 : 128 + CO],
                                start=True, stop=True)
                    for t in range(3):
                        lo0, lo1 = 3 * t, min(3 * t + 3, 8)
                        dst = uhat[:, g0 + lo0 : g0 + lo1]
                        src = pst[t][:, 0 : lo1 - lo0]
                        if (sub4 + t) % 2 == 0:
                            nc.scalar.copy(dst, src)
                        else:
                            nc.vector.tensor_copy(dst, src)
                    for j in range(4):
                        gidx = dc2 * 16 + slot0 + j
                        nc.tensor.matmul(
                            sp1[0:B], lhsT=xc_sb[:, gidx],
                            rhs=xwt_t[:, slot0 + j, 128 : 128 + CO],
                            start=(dc2 == 0 and sub4 == 0 and j == 0),
                            stop=(dc2 == G // 32 - 1 and sub4 == 3
                                  and j == 3))

            # ---------- helpers ----------
            def s_combine(sp, scale):
                # s_sb = (sp0 + sp1 + sp2) * scale; only one PSUM read per op
                nc.scalar.copy(s_sb[:], sp[:, 0])
                nc.vector.tensor_add(s_sb[:], s_sb[:], sp[:, 1])
                nc.vector.tensor_add(s_sb[:], s_sb[:], sp[:, 2])
                if scale != 1.0:
                    nc.scalar.mul(s_sb[:], s_sb[:], scale)

            # ---------- iteration 1 ----------
            # Squash on the 16 real (b) partitions of the s1 accumulator,
            # then replicate v to all (r_l,b) partitions via PE (bmask rows
            # 0:16 are exactly the replication mask; bf16 exact for 0/1).
            nc.scalar.copy(s_sb[0:B], sp1[0:B])
            nc.scalar.mul(s_sb[0:B], s_sb[0:B], 0.1)
            _squash(nc, work, s_sb[0:B], sq[0:B], sq2[0:B], v_bf[0:B],
                    eps_t)
            rep = spsum.tile([128, O, C], F32, tag="sp", name="srep")
            nc.tensor.matmul(rep[:], lhsT=bmask_b[0:B], rhs=v_bf[0:B],
                             start=True, stop=True)
            nc.vector.tensor_copy(v_bf[:], rep[:])

            # ---------- iterations 2..3 ----------
            # All elementwise work stays on DVE (GpSimd contends with DVE
            # for the shared SBUF port pair and is ~3x slower per element,
            # so offloading to it is a net loss). b_ij/c_exp/z_rec are bf16
            # so the small softmax ops run in the 2x DVE perf mode; the
            # errors are per-route and wash out over the 2048-route s-sum.
            for it in range(1, ITERS):
                sp = spsum.tile([128, 3, O, C], F32, tag="sp",
                                name=f"sp_{it}")

                def agree_mult(ch):
                    g0 = ch * CH
                    pa = work.tile([128, CH, O, C], BF16, tag="prod",
                                   name=f"pa{it}_{ch}")
                    nc.vector.tensor_mul(
                        pa[:], uhat[:, g0 : g0 + CH],
                        v_bf[:].unsqueeze(1).broadcast_to((128, CH, O, C)))
                    return pa

                def tree_bij(pa, ch):
                    sl = slice(ch * CH, ch * CH + CH)
                    nc.vector.tensor_add(pa[:, :, 0:8], pa[:, :, 0:8],
                                         pa[:, :, 8:16])
                    nc.vector.tensor_add(pa[:, :, 0:4], pa[:, :, 0:4],
                                         pa[:, :, 4:8])
                    nc.vector.tensor_add(pa[:, :, 0:2], pa[:, :, 0:2],
                                         pa[:, :, 2:4])
                    if it == 1:
                        nc.vector.tensor_add(bij[:, sl],
                                             pa[:, :, 0], pa[:, :, 1])
                    else:
                        nc.vector.tensor_add(pa[:, :, 0], pa[:, :, 0],
                                             pa[:, :, 1])
                        nc.vector.tensor_add(bij[:, sl],
                                             bij[:, sl], pa[:, :, 0])

                def s_prods(ch):
                    sl = slice(ch * CH, ch * CH + CH)
                    prods = work.tile([128, CH, O, C], BF16, tag="prod",
                                      name=f"psx{it}_{ch}")
                    nc.vector.tensor_mul(
                        prods[:], uhat[:, sl],
                        cbf[:, sl, 0:C].unsqueeze(2)
                        .broadcast_to((128, CH, O, C)))
                    # 10 triples + one pair per 32-group chunk
                    for j in range(10):
                        nc.tensor.matmul(
                            sp[:], lhsT=bmask_b[:],
                            rhs=prods[:, 3 * j : 3 * j + 3].rearrange(
                                "p g o c -> p (g o c)"),
                            start=(ch == 0 and j == 0), stop=False)
                    nc.tensor.matmul(
                        sp[:, 0:2], lhsT=bmask_b[:],
                        rhs=prods[:, 30:32].rearrange("p g o c -> p (g o c)"),
                        start=False, stop=(ch == NCH - 1))

                # chunk pairs: softmax ops run on 64-group slices while the
                # next pair's agreement products keep DVE busy behind ACT's
                # exp.
                pa0, pa1 = agree_mult(0), agree_mult(1)
                for cp in range(NCH // 2):
                    ch0 = 2 * cp
                    sl2 = slice(ch0 * CH, ch0 * CH + 2 * CH)
                    tree_bij(pa0, ch0)
                    tree_bij(pa1, ch0 + 1)
                    nc.scalar.activation(cexp[:, sl2], bij[:, sl2],
                                         mybir.ActivationFunctionType.Exp)
                    if ch0 + 2 < NCH:
                        pa0, pa1 = (agree_mult(ch0 + 2),
                                    agree_mult(ch0 + 3))
                    nc.vector.reduce_sum(zsum[:, sl2], cexp[:, sl2],
                                         axis=mybir.AxisListType.X)
                    with nc.allow_low_precision(
                            reason="per-route softmax scale; bf16 error is "
                            "a common factor per route and washes out over "
                            "the 2048-route sum"):
                        nc.vector.reciprocal(zrec[:, sl2], zsum[:, sl2])
                    nc.vector.tensor_mul(
                        cbf[:, sl2, 0:C], cexp[:, sl2],
                        zrec[:, sl2].unsqueeze(2)
                        .broadcast_to((128, 2 * CH, C)))
                    s_prods(ch0)
                    s_prods(ch0 + 1)
                s_combine(sp, 1.0)
                _squash(nc, work, s_sb, sq, sq2, v_bf, eps_t)

            # ---------- output ----------
            vfin = work.tile([128, O, C], F32, tag="vfin")
            nc.vector.tensor_mul(
                vfin[:], s_sb[:],
                sq[:].unsqueeze(1).broadcast_to((128, O, C)))
            nc.sync.dma_start(vout_d[:], vfin[0:B])

    nc.compile()
    return nc


def _squash(nc, work, s_sb, sq, sq2, v16, eps_t):
    """v = s * (|s|^2/(1+|s|^2)) / sqrt(|s|^2 + 1e-8), per (b, c).

    Runs on B=16 partitions; leaves the scale factor in `sq`;
    v16 = s * scale (bf16). s_sb layout (B, O, C).
    """
    P = s_sb.shape[0]
    ssq = work.tile([P, O, C], F32, tag="ssq")
    nc.vector.tensor_mul(ssq[:], s_sb[:], s_sb[:])
    nc.vector.reduce_sum(sq[:], ssq[:].rearrange("p o c -> p c o"),
                         axis=mybir.AxisListType.X)
    # sq2 = (1+n)*sqrt(n+1e-8);  sq = n / sq2
    nc.scalar.activation(sq2[:], sq[:], mybir.ActivationFunctionType.Sqrt,
                         bias=eps_t[0:P])
    nc.vector.scalar_tensor_tensor(
        sq2[:], sq[:], 1.0, sq2[:],
        op0=mybir.AluOpType.add, op1=mybir.AluOpType.mult)
    nc.vector.reciprocal(sq2[:], sq2[:])
    nc.vector.tensor_mul(sq[:], sq[:], sq2[:])
    nc.vector.tensor_mul(
        v16[:], s_sb[:], sq[:].unsqueeze(1).broadcast_to((P, O, C)))


def kernel(x, W):
    global _COMPILED
    shards, xcs, bmask = _host_prep(x, W)
    if _COMPILED is None:
        _COMPILED = _build_kernel()
    nc = _COMPILED
    in_maps = [
        {"xwt": xs, "xc": xc, "bmask": bmask}
        for xs, xc in zip(shards, xcs)
    ]
    res = run_bass_kernel_spmd(nc, in_maps, list(range(N_CORES)))
    outs = []
    for ci in range(N_CORES):
        v = res.results[ci]["vout"]  # (16, O, C)
        outs.append(v.transpose(0, 2, 1))  # -> (16, C, O)
    return np.ascontiguousarray(np.concatenate(outs, axis=0), dtype=np.float32)
